# revision 22
# baseline (speedup 1.0000x reference)
"""2-layer GAT (graph attention) Bass/Tile kernel for Trainium2, 8-core SPMD.

Sharding: nodes partitioned into 6272-row grid-aligned slices (8 x 6272 =
50176 = the 128-padded node grid), edges owned by the dst core, sorted by
dst 128-block, lo/hi-split per block for int16 gather indexing.

Per core and per layer:
  - build the LOCAL feature-table shard ([feat | el | er] columns in one
    matmul per 128-node tile, feature columns interleaved (d, h) so the
    per-head multiplies have packed last dims), AllGather the shard into the
    full [50176, ROW] bf16 table.
  - edge phase per 128-dst block: two dma_gathers (lo/hi) fetch src rows
    [feat | el]; host-precomputed fp8 one-hot matrices give (a) er broadcast
    to edge slots and (b) the dst scatter, both as PE matmuls; attention
    softmax skips max-subtraction (|e| is O(1)); exp and elu run on the
    scalar engine.
All DMAs are batched (few, large, strided) to keep the serialized HWDGE
descriptor queue off the critical path.
"""

import numpy as np
import ml_dtypes

import concourse.bacc as bacc
import concourse.bass as bass
import concourse.mybir as mybir
import concourse.tile as tile
from concourse.masks import make_identity

F32 = mybir.dt.float32
BF16 = mybir.dt.bfloat16
I16 = mybir.dt.int16
FP8 = mybir.dt.float8e4
AF = mybir.ActivationFunctionType
OP = mybir.AluOpType

P = 128
NPBF = ml_dtypes.bfloat16
NPF8 = ml_dtypes.float8_e4m3
ONE8 = np.float32(1.0).astype(NPF8).view(np.uint8)  # fp8 bit pattern of 1.0


class GATCfg:
    def __init__(self, N=50000, C=8, IN=128, HID=32, HEADS=8, OUT=16, NEG=0.2):
        self.N, self.C, self.IN = N, C, IN
        self.HID, self.HEADS, self.OUT, self.NEG = HID, HEADS, OUT, NEG
        self.NP = ((N + C * P - 1) // (C * P)) * (C * P)   # 50176
        self.Nlp = self.NP // C                             # 6272
        self.NB = self.Nlp // P                             # 49
        self.HALF = self.NP // 2                            # tuned by prep
        self.F1 = HEADS * HID                               # 256
        self.F2 = HEADS * OUT                               # 128
        self.ROW1 = 384   # bf16 elems: 768B rows (256B multiple)
        self.ROW2 = 256   # 512B rows
        self.SLO = 0
        self.SHI = 0
        self.GRP = 7      # blocks per load/store group (NB = 7*7)

    @property
    def NCT(self):
        return (self.SLO + self.SHI) // P

    @property
    def NLO(self):
        return self.SLO // P


def prep_indices(src, dst, cfg):
    """Host index preprocessing: per-core per-block slot assignment, wrapped
    int16 gather indices, and fp8 one-hot (scatter + er-broadcast) tensors.
    Shared by both layers (same edges, same node grid)."""
    C, Nlp, NB = cfg.C, cfg.Nlp, cfg.NB
    src = np.asarray(src).astype(np.int64)
    dst = np.asarray(dst).astype(np.int64)
    core = dst // Nlp
    dloc = dst - core * Nlp
    blk = dloc // P
    dblk = dloc - blk * P

    # tune HALF (int16 split point) to minimize gather calls, then padding:
    # calls/block = ceil(SLO/1024) + ceil(SHI/1024) with SLO/SHI the
    # 128-rounded per-block max lo/hi counts.  HALF must keep both index
    # halves < 32768.
    bkey = core * NB + blk
    border = np.argsort(bkey, kind="stable")
    bcnt = np.bincount(bkey, minlength=C * NB)
    bstart = np.zeros(C * NB + 1, np.int64)
    np.cumsum(bcnt, out=bstart[1:])
    ssrc = src[border]
    srt = np.zeros_like(ssrc)
    for k in range(C * NB):
        seg = np.sort(ssrc[bstart[k]:bstart[k + 1]])
        srt[bstart[k]:bstart[k + 1]] = seg
    best = None
    for half in range(cfg.NP - 32768, 32768 + 128, 128):
        lo_max = 0
        hi_max = 0
        for k in range(C * NB):
            seg = srt[bstart[k]:bstart[k + 1]]
            nlo = int(np.searchsorted(seg, half))
            lo_max = max(lo_max, nlo)
            hi_max = max(hi_max, seg.size - nlo)
        SLO = max(P, ((lo_max + P - 1) // P) * P)
        SHI = ((hi_max + P - 1) // P) * P
        calls = -(-SLO // 1024) + -(-SHI // 1024)
        score = (calls, SLO + SHI)
        if best is None or score < best[0]:
            best = (score, half, SLO, SHI)
    _, HALF, SLO, SHI = best
    cfg.HALF, cfg.SLO, cfg.SHI = HALF, SLO, SHI
    is_hi = src >= HALF

    # order edges by (core, block, is_hi); compute slot-in-group
    key = (core * NB + blk) * 2 + is_hi
    order = np.argsort(key, kind="stable")
    kord = key[order]
    counts = np.bincount(kord, minlength=C * NB * 2)
    starts = np.zeros(C * NB * 2 + 1, np.int64)
    np.cumsum(counts, out=starts[1:])
    pos_in_grp = np.arange(len(order)) - starts[kord]
    S = SLO + SHI
    NCT = S // P
    S16 = S // 16

    e = order
    ecore = core[e]
    eblk = blk[e]
    edb = dblk[e]
    ehi = is_hi[e]
    eslot = pos_in_grp + np.where(ehi, SLO, 0)
    eidx = np.where(ehi, src[e] - HALF, src[e])

    # slot-ordered index values [C, NB, S]
    vals = np.zeros((C, NB, S), np.int64)
    vals[ecore, eblk, eslot] = eidx
    # wrap16: [C, NB, S] -> [C, 128, NB*S16] (16-wrapped, replicated 8x)
    w = vals.reshape(C, NB, S16, 16).transpose(0, 3, 1, 2)  # [C,16,NB,S16]
    w = w.reshape(C, 16, NB * S16).astype(np.int16)
    idxw = np.tile(w, (1, 8, 1))                            # [C,128,NB*S16]

    # fp8 one-hots [C, 128, NB, 2, NCT, 128]
    ohz = np.zeros((C, P, NB, 2, NCT, P), np.uint8)
    ech = eslot // P
    epp = eslot - ech * P
    ohz[ecore, epp, eblk, 0, ech, edb] = ONE8
    ohz[ecore, edb, eblk, 1, ech, epp] = ONE8
    return {"idx": idxw, "ohz": ohz.view(NPF8)}


def _perm_dh(H, D):
    """Column permutation (h, d) -> (d, h): newcol[d*H+h] = oldcol[h*D+d]."""
    pm = np.zeros(H * D, np.int64)
    for h in range(H):
        for d in range(D):
            pm[d * H + h] = h * D + d
    return pm


def host_inputs(inputs, cfg, idx):
    N, C, IN = cfg.N, cfg.C, cfg.IN
    H, D1, D2 = cfg.HEADS, cfg.HID, cfg.OUT
    F1, F2, Nlp = cfg.F1, cfg.F2, cfg.Nlp
    x = np.asarray(inputs["x"], np.float32)
    W1 = np.asarray(inputs["W1"], np.float32)
    W2 = np.asarray(inputs["W2"], np.float32)
    al1 = np.asarray(inputs["al1"], np.float32)
    ar1 = np.asarray(inputs["ar1"], np.float32)
    al2 = np.asarray(inputs["al2"], np.float32)
    ar2 = np.asarray(inputs["ar2"], np.float32)
    b1 = np.asarray(inputs["b1"], np.float32)
    b2 = np.asarray(inputs["b2"], np.float32)

    p1 = _perm_dh(H, D1)
    p2 = _perm_dh(H, D2)
    Wel1 = np.einsum("ihd,hd->ih", W1.reshape(IN, H, D1), al1)
    Wer1 = np.einsum("ihd,hd->ih", W1.reshape(IN, H, D1), ar1)
    RHS1 = np.concatenate([W1[:, p1], Wel1, Wer1], axis=1).astype(NPBF)

    W2p = W2[p1][:, p2]                       # rows (d,h), cols (o,h2)
    Wel2 = np.einsum("rhd,hd->rh", W2.reshape(F1, H, D2), al2)[p1]
    Wer2 = np.einsum("rhd,hd->rh", W2.reshape(F1, H, D2), ar2)[p1]
    RHS2 = np.concatenate([W2p, Wel2, Wer2], axis=1)       # [256, 144]
    RHS2 = RHS2.reshape(2, P, F2 + 16).astype(NPBF)

    B1M = np.broadcast_to(b1[p1][None, :], (P, F1)).copy()
    b2m = b2.reshape(H, D2).mean(axis=0)
    B2M = np.broadcast_to(b2m[None, :], (P, D2)).copy()

    xT = np.zeros((IN, cfg.NP), np.float32)
    xT[:, :N] = x.T
    XT = xT.astype(NPBF)

    in_maps = []
    for c in range(C):
        in_maps.append({
            "XT": XT,
            "xTl": XT[:, c * Nlp:(c + 1) * Nlp].copy(),
            "RHS1": RHS1, "RHS2": RHS2, "B1M": B1M, "B2M": B2M,
            "IDX": idx["idx"][c], "OHZ": idx["ohz"][c],
        })
    return in_maps


def build_module(cfg, dbg=False, skip_cc=False):
    nc = bacc.Bacc("TRN2", target_bir_lowering=False, debug=False,
                   num_devices=cfg.C)
    C, NB, Nlp, NP = cfg.C, cfg.NB, cfg.Nlp, cfg.NP
    F1, F2, ROW1, ROW2 = cfg.F1, cfg.F2, cfg.ROW1, cfg.ROW2
    SLO, SHI, NCT, NLO = cfg.SLO, cfg.SHI, cfg.NCT, cfg.NLO
    GRP = cfg.GRP
    NG = NB // GRP
    S16 = (SLO + SHI) // 16
    L16 = SLO // 16
    H16 = SHI // 16
    OUTW = cfg.OUT

    d_XT = nc.dram_tensor("XT", [cfg.IN, NP], BF16, kind="ExternalInput")
    d_xTl = nc.dram_tensor("xTl", [cfg.IN, Nlp], BF16, kind="ExternalInput")
    d_RHS1 = nc.dram_tensor("RHS1", [cfg.IN, F1 + 16], BF16,
                            kind="ExternalInput")
    d_RHS2 = nc.dram_tensor("RHS2", [2, P, F2 + 16], BF16,
                            kind="ExternalInput")
    d_B1M = nc.dram_tensor("B1M", [P, F1], F32, kind="ExternalInput")
    d_B2M = nc.dram_tensor("B2M", [P, OUTW], F32, kind="ExternalInput")
    d_IDX = nc.dram_tensor("IDX", [P, NB * S16], I16, kind="ExternalInput")
    d_OHZ = nc.dram_tensor("OHZ", [P, NB, 2, NCT, P], FP8,
                           kind="ExternalInput")
    d_out = nc.dram_tensor("out", [Nlp, OUTW], F32, kind="ExternalOutput")

    shared = "Shared" if C > 4 else "Local"
    d_tab1 = nc.dram_tensor("tab1", [NP, ROW1], BF16, kind="Internal")
    d_t2loc = nc.dram_tensor("t2loc", [Nlp, ROW2], BF16, kind="Internal")
    d_tab2 = nc.dram_tensor("tab2", [NP, ROW2], BF16, kind="Internal",
                            addr_space=shared)
    d_er1 = nc.dram_tensor("er1", [NB, P, 8], BF16, kind="Internal")
    d_er2 = nc.dram_tensor("er2", [NB, P, 8], BF16, kind="Internal")

    with tile.TileContext(nc) as tc:
        with (
            tc.tile_pool(name="const", bufs=1) as cpool,
            tc.tile_pool(name="work", bufs=3) as wpool,
            tc.tile_pool(name="gath", bufs=3) as gpool,
        ):
            ident = cpool.tile([P, P], BF16)
            make_identity(nc, ident[:])
            rhs1 = cpool.tile([P, F1 + 16], BF16)
            nc.sync.dma_start(rhs1[:], d_RHS1[:, :])
            rhs2 = cpool.tile([P, 2, F2 + 16], BF16)
            nc.sync.dma_start(rhs2[:],
                              d_RHS2[:, :, :].rearrange("q p x -> p q x"))
            b1m = cpool.tile([P, F1], F32)
            nc.sync.dma_start(b1m[:], d_B1M[:, :])
            b2m = cpool.tile([P, OUTW], F32)
            nc.sync.dma_start(b2m[:], d_B2M[:, :])
            hTs = cpool.tile([P, 2, Nlp], BF16)   # persistent h^T (layer 1)

            # ---------------- tab1: replicated full-table build ----------
            GRP1 = 14
            NT1 = NP // P                       # 392 tiles, 28 groups
            with tc.tile_pool(name="t1ps", bufs=4, space="PSUM") as t1ps:
                for g in range(NT1 // GRP1):
                    xg = wpool.tile([P, GRP1, P], BF16, tag="xg")
                    nc.sync.dma_start(
                        xg[:], d_XT[:, g * GRP1 * P:(g + 1) * GRP1 * P]
                        .rearrange("p (t c) -> p t c", t=GRP1))
                    feg = wpool.tile([P, GRP1, F1 + 8], BF16, tag="feg1")
                    for t in range(GRP1):
                        ps = t1ps.tile([P, F1 + 8], F32, tag="tbl")
                        nc.tensor.matmul(ps[:], lhsT=xg[:, t, :],
                                         rhs=rhs1[:, 0:F1 + 8], start=True,
                                         stop=True)
                        (nc.vector.tensor_copy if t % 2 == 0
                         else nc.scalar.copy)(feg[:, t, :], ps[:])
                    r0 = g * GRP1 * P
                    nc.sync.dma_start(
                        d_tab1[r0:r0 + GRP1 * P, 0:F1 + 8]
                        .rearrange("(t p) x -> p t x", p=P),
                        feg[:, :, :])
                # er1 for local rows (from xTl)
                for g in range(NG):
                    xlg = wpool.tile([P, GRP, P], BF16, tag="xlg")
                    nc.sync.dma_start(
                        xlg[:], d_xTl[:, g * GRP * P:(g + 1) * GRP * P]
                        .rearrange("p (t c) -> p t c", t=GRP))
                    erg1 = wpool.tile([P, GRP, 8], BF16, tag="erg1")
                    for t in range(GRP):
                        ps = t1ps.tile([P, 8], F32, tag="er1")
                        nc.tensor.matmul(ps[:], lhsT=xlg[:, t, :],
                                         rhs=rhs1[:, F1 + 8:F1 + 16],
                                         start=True, stop=True)
                        nc.vector.tensor_copy(erg1[:, t, :], ps[:])
                    nc.sync.dma_start(
                        d_er1[g * GRP:(g + 1) * GRP, :, :]
                        .rearrange("t p x -> p t x"),
                        erg1[:])

            # ---------------- tab2 shard build ----------------
            def table_phase2(tps):
                FE = F2 + 8
                for g in range(NG):
                    feg = wpool.tile([P, GRP, FE + 8], BF16, tag="feg2")
                    for t in range(GRP):
                        ps = tps.tile([P, FE + 8], F32, tag="tbl")
                        tr = slice((g * GRP + t) * P, (g * GRP + t + 1) * P)
                        nc.tensor.matmul(ps[:], lhsT=hTs[:, 0, tr],
                                         rhs=rhs2[:, 0, :], start=True,
                                         stop=False)
                        nc.tensor.matmul(ps[:], lhsT=hTs[:, 1, tr],
                                         rhs=rhs2[:, 1, :], start=False,
                                         stop=True)
                        (nc.vector.tensor_copy if t % 2 == 0
                         else nc.scalar.copy)(feg[:, t, :], ps[:])
                    r0 = g * GRP * P
                    nc.sync.dma_start(
                        d_t2loc[r0:r0 + GRP * P, 0:FE]
                        .rearrange("(t p) x -> p t x", p=P),
                        feg[:, :, 0:FE])
                    nc.sync.dma_start(
                        d_er2[g * GRP:(g + 1) * GRP, :, :]
                        .rearrange("t p x -> p t x"),
                        feg[:, :, FE:FE + 8])

            # ---------------- edge phase (shared) ----------------
            def edge_phase(layer, pspool, tps, finalize):
                F = F1 if layer == 1 else F2
                ROW = ROW1 if layer == 1 else ROW2
                DW = 32 if layer == 1 else 16
                tab = d_tab1 if layer == 1 else d_tab2
                d_er = d_er1 if layer == 1 else d_er2
                pending = None
                for g in range(NG):
                    ixg = wpool.tile([P, GRP, S16], I16, tag="ixg")
                    nc.sync.dma_start(
                        ixg[:], d_IDX[:, g * GRP * S16:(g + 1) * GRP * S16]
                        .rearrange("p (t s) -> p t s", t=GRP))
                    erg = wpool.tile([P, GRP, 8], BF16, tag="erg")
                    nc.sync.dma_start(
                        erg[:], d_er[g * GRP:(g + 1) * GRP, :, :]
                        .rearrange("t p x -> p t x"))
                    for j in range(GRP):
                        b = g * GRP + j
                        oz = gpool.tile([P, 2, NCT, P], FP8, tag="oz")
                        nc.sync.dma_start(oz[:], d_OHZ[:, b, :, :, :])
                        G = gpool.tile([P, NCT, ROW], BF16, tag=f"G{layer}")
                        PIECE = 1024  # 64-desc/lane packet cap
                        for s0 in range(0, SLO, PIECE):
                            n = min(PIECE, SLO - s0)
                            nc.gpsimd.dma_gather(
                                out_ap=G[:, s0 // P:(s0 + n) // P, :],
                                in_ap=tab[:, :],
                                idxs_ap=ixg[:, j, s0 // 16:(s0 + n) // 16],
                                num_idxs=n, num_idxs_reg=n, elem_size=ROW)
                        for s0 in range(SLO, SLO + SHI, PIECE):
                            n = min(PIECE, SLO + SHI - s0)
                            nc.gpsimd.dma_gather(
                                out_ap=G[:, s0 // P:(s0 + n) // P, :],
                                in_ap=tab[cfg.HALF:NP, :],
                                idxs_ap=ixg[:, j, s0 // 16:(s0 + n) // 16],
                                num_idxs=n, num_idxs_reg=n, elem_size=ROW)
                        erps = tps.tile([P, NCT, 8], F32, tag="erps")
                        for c in range(NCT):
                            nc.tensor.matmul(erps[:, c, :],
                                             lhsT=oz[:, 1, c, :],
                                             rhs=erg[:, j, :],
                                             start=True, stop=True)
                        ea = wpool.tile([P, NCT, 8], F32, tag="ea")
                        nc.vector.tensor_tensor(out=ea[:],
                                                in0=G[:, :, F:F + 8],
                                                in1=erps[:], op=OP.add)
                        es = wpool.tile([P, NCT, 8], F32, tag="es")
                        nc.vector.tensor_scalar(es[:], ea[:], cfg.NEG, None,
                                                op0=OP.mult)
                        nc.vector.tensor_tensor(out=es[:], in0=ea[:],
                                                in1=es[:], op=OP.max)
                        nc.scalar.activation(G[:, :, F:F + 8], es[:], AF.Exp)
                        # interleave the exp-multiply and the scatter matmuls
                        # in ~NCT/3-chunk groups so the PE queue never stalls
                        # behind the whole multiply
                        ps = pspool.tile([P, F + 8], F32, tag="eps")
                        splits = [0, NCT // 3, 2 * NCT // 3, NCT]
                        for c0, c1 in zip(splits[:-1], splits[1:]):
                            nc.vector.tensor_tensor(
                                out=G[:, c0:c1, 0:F].rearrange(
                                    "p c (d h) -> p c d h", h=8),
                                in0=G[:, c0:c1, 0:F].rearrange(
                                    "p c (d h) -> p c d h", h=8),
                                in1=G[:, c0:c1, F:F + 8].rearrange(
                                    "p c (one h) -> p c one h", one=1)
                                    .to_broadcast([P, c1 - c0, DW, 8]),
                                op=OP.mult)
                            for c in range(c0, c1):
                                nc.tensor.matmul(ps[:], lhsT=oz[:, 0, c, :],
                                                 rhs=G[:, c, 0:F + 8],
                                                 start=(c == 0),
                                                 stop=(c == NCT - 1))
                        esum = wpool.tile([P, 8], F32, tag="esum")
                        # layer 2 folds the head-mean 1/8 into the reciprocal
                        if layer == 2:
                            nc.vector.tensor_scalar(esum[:], ps[:, F:F + 8],
                                                    1e-30, 8.0, op0=OP.max,
                                                    op1=OP.mult)
                        else:
                            nc.vector.tensor_scalar(esum[:], ps[:, F:F + 8],
                                                    1e-30, None, op0=OP.max)
                        inv = wpool.tile([P, 8], F32, tag="inv")
                        nc.vector.reciprocal(inv[:], esum[:])
                        if pending is not None:
                            finalize(*pending)
                        pending = (b, ps, inv)
                if pending is not None:
                    finalize(*pending)

            # ---------------- layer-1 edges -> hTs ----------------
            with tc.tile_pool(name="e1ps", bufs=3, space="PSUM") as e1ps, \
                 tc.tile_pool(name="tps", bufs=2, space="PSUM") as tps:
                def fin1(b, ps, inv):
                    z = wpool.tile([P, F1], F32, tag="z")
                    nc.vector.tensor_tensor(
                        out=z[:].rearrange("p (d h) -> p d h", h=8),
                        in0=ps[:, 0:F1].rearrange("p (d h) -> p d h", h=8),
                        in1=inv[:].rearrange("p (one h) -> p one h", one=1)
                            .to_broadcast([P, 32, 8]),
                        op=OP.mult)
                    zb = wpool.tile([P, F1], F32, tag="zb")
                    nc.vector.tensor_tensor(out=zb[:], in0=z[:], in1=b1m[:],
                                            op=OP.add)
                    zm = wpool.tile([P, F1], F32, tag="zm")
                    nc.vector.tensor_scalar(zm[:], zb[:], 0.0, None,
                                            op0=OP.min)
                    eb = wpool.tile([P, F1], BF16, tag="eb")
                    nc.scalar.activation(eb[:], zm[:], AF.Exp)
                    rb = wpool.tile([P, F1], BF16, tag="rb")
                    nc.scalar.activation(rb[:], zb[:], AF.Relu)
                    hs = wpool.tile([P, F1], BF16, tag="hs")
                    nc.vector.tensor_tensor(out=hs[:], in0=eb[:], in1=rb[:],
                                            op=OP.add)
                    hm = wpool.tile([P, F1], BF16, tag="hm")
                    nc.vector.tensor_scalar(hm[:], hs[:], -1.0, None,
                                            op0=OP.add)
                    for q in range(2):
                        pst = tps.tile([P, P], BF16, tag="pst")
                        nc.tensor.transpose(pst[:], hm[:, q * P:(q + 1) * P],
                                            ident[:])
                        (nc.vector.tensor_copy if q == 0
                         else nc.scalar.copy)(
                            hTs[:, q, b * P:(b + 1) * P], pst[:])

                edge_phase(1, e1ps, tps, fin1)

            # ---------------- layer-2 table + AG ----------------
            with tc.tile_pool(name="t2ps", bufs=3, space="PSUM") as t2ps:
                table_phase2(t2ps)
            if not skip_cc:
                nc.gpsimd.collective_compute(
                    "AllGather", OP.bypass,
                    replica_groups=[list(range(C))],
                    ins=[d_t2loc[:, :]], outs=[d_tab2[:, :]])

            # ---------------- layer-2 edges -> out ----------------
            with tc.tile_pool(name="e2ps", bufs=3, space="PSUM") as e2ps, \
                 tc.tile_pool(name="tps2", bufs=2, space="PSUM") as tps2:
                ogbox = [None]

                def fin2(b, ps, inv):
                    if b % GRP == 0:
                        ogbox[0] = wpool.tile([P, GRP, OUTW], F32, tag="og",
                                              name="og")
                    og = ogbox[0]
                    w_ = wpool.tile([P, F2], F32, tag="w_")
                    nc.vector.tensor_tensor(
                        out=w_[:].rearrange("p (o h) -> p o h", h=8),
                        in0=ps[:, 0:F2].rearrange("p (o h) -> p o h", h=8),
                        in1=inv[:].rearrange("p (one h) -> p one h", one=1)
                            .to_broadcast([P, OUTW, 8]),
                        op=OP.mult)
                    ws = wpool.tile([P, OUTW], F32, tag="ws")
                    nc.vector.tensor_reduce(
                        ws[:], w_[:].rearrange("p (o h) -> p o h", h=8),
                        axis=mybir.AxisListType.X, op=OP.add)
                    nc.vector.tensor_tensor(out=og[:, b % GRP, :], in0=ws[:],
                                            in1=b2m[:], op=OP.add)
                    if b % GRP == GRP - 1:
                        r0 = (b - GRP + 1) * P
                        nc.sync.dma_start(
                            d_out[r0:r0 + GRP * P, :]
                            .rearrange("(t p) x -> p t x", p=P),
                            og[:, :, :])

                edge_phase(2, e2ps, tps2, fin2)

            if dbg:
                for nm, src_t in [("dbg_t1loc", d_t1loc), ("dbg_er1", d_er1),
                                  ("dbg_tab1", d_tab1), ("dbg_t2loc", d_t2loc),
                                  ("dbg_er2", d_er2), ("dbg_tab2", d_tab2)]:
                    dd = nc.dram_tensor(nm, list(src_t.shape), BF16,
                                        kind="ExternalOutput")
                    sl = tuple(slice(None) for _ in src_t.shape)
                    nc.sync.dma_start(dd[sl], src_t[sl])

    nc.compile()
    return nc


# ----------------------------------------------------------------------------
_CACHE = {}


def get_built(src, dst, C=8, cfg=None):
    key = (hash(src.tobytes()), hash(dst.tobytes()), C)
    if key not in _CACHE:
        if cfg is None:
            cfg = GATCfg(C=C)
        idx = prep_indices(src, dst, cfg)
        nc = build_module(cfg)
        _CACHE[key] = (cfg, idx, nc)
    return _CACHE[key]


_EXECC = {}


def _get_exec(key, nc, n_cores):
    """Persistent jit(shard_map(bass_exec)) so repeated kernel() calls skip
    retracing/recompiling."""
    if key in _EXECC:
        return _EXECC[key]
    import jax
    from jax.experimental.shard_map import shard_map
    from jax.sharding import Mesh, NamedSharding, PartitionSpec
    from concourse import bass2jax
    bass2jax.install_neuronx_cc_hook()
    partition_name = (nc.partition_id_tensor.name
                      if nc.partition_id_tensor else None)
    in_names, out_names, out_avals, zero_shapes = [], [], [], []
    for alloc in nc.m.functions[0].allocations:
        if not isinstance(alloc, mybir.MemoryLocationSet):
            continue
        name = alloc.memorylocations[0].name
        if alloc.kind == "ExternalInput":
            if name != partition_name:
                in_names.append(name)
        elif alloc.kind == "ExternalOutput":
            out_names.append(name)
            shape = tuple(alloc.tensor_shape)
            dtype = mybir.dt.np(alloc.dtype)
            out_avals.append(jax.core.ShapedArray(shape, dtype))
            zero_shapes.append((shape, dtype))
    n_params = len(in_names)
    in_names_all = list(in_names) + out_names + (
        [partition_name] if partition_name else [])

    def _body(*args):
        ops = list(args)
        if partition_name:
            ops.append(bass2jax.partition_id_tensor())
        outs = bass2jax._bass_exec_p.bind(
            *ops, out_avals=tuple(out_avals), in_names=tuple(in_names_all),
            out_names=tuple(out_names), lowering_input_output_aliases=(),
            sim_require_finite=True, sim_require_nnan=True, nc=nc)
        return tuple(outs)

    devices = jax.devices()[:n_cores]
    mesh = Mesh(np.asarray(devices), ("core",))
    nout = len(out_names)
    f = jax.jit(shard_map(
        _body, mesh=mesh,
        in_specs=(PartitionSpec("core"),) * (n_params + nout),
        out_specs=(PartitionSpec("core"),) * nout, check_rep=False),
        keep_unused=True)
    sh = NamedSharding(mesh, PartitionSpec("core"))
    ent = dict(f=f, in_names=in_names, out_names=out_names,
               zero_shapes=zero_shapes, sh=sh, argcache=None)
    _EXECC[key] = ent
    return ent


def kernel(**inputs) -> np.ndarray:
    import jax
    src = np.asarray(inputs["src"], np.int32)
    dst = np.asarray(inputs["dst"], np.int32)
    x = np.asarray(inputs["x"])
    base = GATCfg(N=int(x.shape[0]), C=8, IN=int(x.shape[1]))
    cfg, idx, nc = get_built(src, dst, C=8, cfg=base)
    in_maps = host_inputs(inputs, cfg, idx)
    key = (hash(src.tobytes()), hash(dst.tobytes()), cfg.C)
    ent = _get_exec(key, nc, cfg.C)
    C = cfg.C
    concat_in = [np.ascontiguousarray(
        np.concatenate([in_maps[c][nm] for c in range(C)], axis=0))
        for nm in ent["in_names"]]
    hashes = tuple(hash(a.tobytes()) for a in concat_in)
    if ent["argcache"] is None or ent["argcache"][0] != hashes:
        zeros = [np.zeros((C * sh0[0], *sh0[1:]), dt)
                 for sh0, dt in ent["zero_shapes"]]
        args = [jax.device_put(a, ent["sh"]) for a in concat_in + zeros]
        ent["argcache"] = (hashes, args)
    args = ent["argcache"][1]
    outs = ent["f"](*args)
    jax.block_until_ready(outs)
    oi = ent["out_names"].index("out")
    out = np.asarray(outs[oi]).reshape(C, cfg.Nlp, cfg.OUT)
    return out.reshape(-1, cfg.OUT)[:cfg.N].astype(np.float32)


# revision 33
# speedup vs baseline: 1.0627x; 1.0627x over previous
"""2-layer GAT (graph attention) Bass/Tile kernel for Trainium2, 8-core SPMD.

Sharding: nodes partitioned into 6272-row grid-aligned slices (8 x 6272 =
50176 = the 128-padded node grid), edges owned by the dst core, sorted by
dst 128-block, lo/hi-split per block for int16 gather indexing.

Per core and per layer:
  - build the LOCAL feature-table shard ([feat | el | er] columns in one
    matmul per 128-node tile, feature columns interleaved (d, h) so the
    per-head multiplies have packed last dims), AllGather the shard into the
    full [50176, ROW] bf16 table.
  - edge phase per 128-dst block: two dma_gathers (lo/hi) fetch src rows
    [feat | el]; host-precomputed fp8 one-hot matrices give (a) er broadcast
    to edge slots and (b) the dst scatter, both as PE matmuls; attention
    softmax skips max-subtraction (|e| is O(1)); exp and elu run on the
    scalar engine.
All DMAs are batched (few, large, strided) to keep the serialized HWDGE
descriptor queue off the critical path.
"""

import numpy as np
import ml_dtypes

import concourse.bacc as bacc
import concourse.bass as bass
import concourse.mybir as mybir
import concourse.tile as tile
from concourse.masks import make_identity

F32 = mybir.dt.float32
BF16 = mybir.dt.bfloat16
I16 = mybir.dt.int16
FP8 = mybir.dt.float8e4
AF = mybir.ActivationFunctionType
OP = mybir.AluOpType

P = 128
NPBF = ml_dtypes.bfloat16
NPF8 = ml_dtypes.float8_e4m3
ONE8 = np.float32(1.0).astype(NPF8).view(np.uint8)  # fp8 bit pattern of 1.0


class GATCfg:
    def __init__(self, N=50000, C=8, IN=128, HID=32, HEADS=8, OUT=16, NEG=0.2):
        self.N, self.C, self.IN = N, C, IN
        self.HID, self.HEADS, self.OUT, self.NEG = HID, HEADS, OUT, NEG
        self.NP = ((N + C * P - 1) // (C * P)) * (C * P)   # 50176
        self.Nlp = self.NP // C                             # 6272
        self.NB = self.Nlp // P                             # 49
        self.HALF = self.NP // 2                            # tuned by prep
        self.F1 = HEADS * HID                               # 256
        self.F2 = HEADS * OUT                               # 128
        self.ROW1 = 384   # bf16 elems: 768B rows (256B multiple)
        self.ROW2 = 256   # 512B rows
        self.SLO = 0
        self.SHI = 0
        self.GRP = 7      # blocks per load/store group (NB = 7*7)

    @property
    def NCT(self):
        return (self.SLO + self.SHI) // P

    @property
    def NLO(self):
        return self.SLO // P


def prep_indices(src, dst, cfg):
    """Host index preprocessing: per-core per-block slot assignment, wrapped
    int16 gather indices, and fp8 one-hot (scatter + er-broadcast) tensors.
    Shared by both layers (same edges, same node grid)."""
    C, Nlp, NB = cfg.C, cfg.Nlp, cfg.NB
    src = np.asarray(src).astype(np.int64)
    dst = np.asarray(dst).astype(np.int64)
    core = dst // Nlp
    dloc = dst - core * Nlp
    blk = dloc // P
    dblk = dloc - blk * P

    # tune HALF (int16 split point) to minimize gather calls, then padding:
    # calls/block = ceil(SLO/1024) + ceil(SHI/1024) with SLO/SHI the
    # 128-rounded per-block max lo/hi counts.  HALF must keep both index
    # halves < 32768.
    bkey = core * NB + blk
    border = np.argsort(bkey, kind="stable")
    bcnt = np.bincount(bkey, minlength=C * NB)
    bstart = np.zeros(C * NB + 1, np.int64)
    np.cumsum(bcnt, out=bstart[1:])
    ssrc = src[border]
    srt = np.zeros_like(ssrc)
    for k in range(C * NB):
        seg = np.sort(ssrc[bstart[k]:bstart[k + 1]])
        srt[bstart[k]:bstart[k + 1]] = seg
    best = None
    for half in range(cfg.NP - 32768, 32768 + 128, 128):
        lo_max = 0
        hi_max = 0
        for k in range(C * NB):
            seg = srt[bstart[k]:bstart[k + 1]]
            nlo = int(np.searchsorted(seg, half))
            lo_max = max(lo_max, nlo)
            hi_max = max(hi_max, seg.size - nlo)
        SLO = max(P, ((lo_max + P - 1) // P) * P)
        SHI = ((hi_max + P - 1) // P) * P
        calls = -(-SLO // 1024) + -(-SHI // 1024)
        score = (calls, SLO + SHI)
        if best is None or score < best[0]:
            best = (score, half, SLO, SHI)
    _, HALF, SLO, SHI = best
    cfg.HALF, cfg.SLO, cfg.SHI = HALF, SLO, SHI
    is_hi = src >= HALF

    # order edges by (core, block, is_hi); compute slot-in-group
    key = (core * NB + blk) * 2 + is_hi
    order = np.argsort(key, kind="stable")
    kord = key[order]
    counts = np.bincount(kord, minlength=C * NB * 2)
    starts = np.zeros(C * NB * 2 + 1, np.int64)
    np.cumsum(counts, out=starts[1:])
    pos_in_grp = np.arange(len(order)) - starts[kord]
    S = SLO + SHI
    NCT = S // P
    S16 = S // 16

    e = order
    ecore = core[e]
    eblk = blk[e]
    edb = dblk[e]
    ehi = is_hi[e]
    eslot = pos_in_grp + np.where(ehi, SLO, 0)
    eidx = np.where(ehi, src[e] - HALF, src[e])

    # slot-ordered index values [C, NB, S]
    vals = np.zeros((C, NB, S), np.int64)
    vals[ecore, eblk, eslot] = eidx
    # wrap16: [C, NB, S] -> [C, 128, NB*S16] (16-wrapped, replicated 8x)
    w = vals.reshape(C, NB, S16, 16).transpose(0, 3, 1, 2)  # [C,16,NB,S16]
    w = w.reshape(C, 16, NB * S16).astype(np.int16)
    idxw = np.tile(w, (1, 8, 1))                            # [C,128,NB*S16]

    # fp8 one-hots [C, 128, NB, 2, NCT, 128]
    ohz = np.zeros((C, P, NB, 2, NCT, P), np.uint8)
    ech = eslot // P
    epp = eslot - ech * P
    ohz[ecore, epp, eblk, 0, ech, edb] = ONE8
    ohz[ecore, edb, eblk, 1, ech, epp] = ONE8
    return {"idx": idxw, "ohz": ohz.view(NPF8)}


def _perm_dh(H, D):
    """Column permutation (h, d) -> (d, h): newcol[d*H+h] = oldcol[h*D+d]."""
    pm = np.zeros(H * D, np.int64)
    for h in range(H):
        for d in range(D):
            pm[d * H + h] = h * D + d
    return pm


def host_inputs(inputs, cfg, idx):
    N, C, IN = cfg.N, cfg.C, cfg.IN
    H, D1, D2 = cfg.HEADS, cfg.HID, cfg.OUT
    F1, F2, Nlp = cfg.F1, cfg.F2, cfg.Nlp
    x = np.asarray(inputs["x"], np.float32)
    W1 = np.asarray(inputs["W1"], np.float32)
    W2 = np.asarray(inputs["W2"], np.float32)
    al1 = np.asarray(inputs["al1"], np.float32)
    ar1 = np.asarray(inputs["ar1"], np.float32)
    al2 = np.asarray(inputs["al2"], np.float32)
    ar2 = np.asarray(inputs["ar2"], np.float32)
    b1 = np.asarray(inputs["b1"], np.float32)
    b2 = np.asarray(inputs["b2"], np.float32)

    p1 = _perm_dh(H, D1)
    p2 = _perm_dh(H, D2)
    Wel1 = np.einsum("ihd,hd->ih", W1.reshape(IN, H, D1), al1)
    Wer1 = np.einsum("ihd,hd->ih", W1.reshape(IN, H, D1), ar1)
    RHS1 = np.concatenate([W1[:, p1], Wel1, Wer1], axis=1).astype(NPBF)

    W2p = W2[p1][:, p2]                       # rows (d,h), cols (o,h2)
    Wel2 = np.einsum("rhd,hd->rh", W2.reshape(F1, H, D2), al2)[p1]
    Wer2 = np.einsum("rhd,hd->rh", W2.reshape(F1, H, D2), ar2)[p1]
    RHS2 = np.concatenate([W2p, Wel2, Wer2], axis=1)       # [256, 144]
    RHS2 = RHS2.reshape(2, P, F2 + 16).astype(NPBF)

    B1M = np.broadcast_to(b1[p1][None, :], (P, F1)).copy()
    b2m = b2.reshape(H, D2).mean(axis=0)
    B2M = np.broadcast_to(b2m[None, :], (P, D2)).copy()

    xT = np.zeros((IN, cfg.NP), np.float32)
    xT[:, :N] = x.T
    XT = xT.astype(NPBF)

    in_maps = []
    for c in range(C):
        in_maps.append({
            "XT": XT,
            "xTl": XT[:, c * Nlp:(c + 1) * Nlp].copy(),
            "RHS1": RHS1, "RHS2": RHS2, "B1M": B1M, "B2M": B2M,
            "IDX": idx["idx"][c], "OHZ": idx["ohz"][c],
        })
    return in_maps


def build_module(cfg, dbg=False, skip_cc=False):
    nc = bacc.Bacc("TRN2", target_bir_lowering=False, debug=False,
                   num_devices=cfg.C)
    C, NB, Nlp, NP = cfg.C, cfg.NB, cfg.Nlp, cfg.NP
    F1, F2, ROW1, ROW2 = cfg.F1, cfg.F2, cfg.ROW1, cfg.ROW2
    SLO, SHI, NCT, NLO = cfg.SLO, cfg.SHI, cfg.NCT, cfg.NLO
    GRP = cfg.GRP
    NG = NB // GRP
    S16 = (SLO + SHI) // 16
    L16 = SLO // 16
    H16 = SHI // 16
    OUTW = cfg.OUT

    d_XT = nc.dram_tensor("XT", [cfg.IN, NP], BF16, kind="ExternalInput")
    d_xTl = nc.dram_tensor("xTl", [cfg.IN, Nlp], BF16, kind="ExternalInput")
    d_RHS1 = nc.dram_tensor("RHS1", [cfg.IN, F1 + 16], BF16,
                            kind="ExternalInput")
    d_RHS2 = nc.dram_tensor("RHS2", [2, P, F2 + 16], BF16,
                            kind="ExternalInput")
    d_B1M = nc.dram_tensor("B1M", [P, F1], F32, kind="ExternalInput")
    d_B2M = nc.dram_tensor("B2M", [P, OUTW], F32, kind="ExternalInput")
    d_IDX = nc.dram_tensor("IDX", [P, NB * S16], I16, kind="ExternalInput")
    d_OHZ = nc.dram_tensor("OHZ", [P, NB, 2, NCT, P], FP8,
                           kind="ExternalInput")
    d_out = nc.dram_tensor("out", [Nlp, OUTW], F32, kind="ExternalOutput")

    shared = "Shared" if C > 4 else "Local"
    d_t1loc = nc.dram_tensor("t1loc", [Nlp, ROW1], BF16, kind="Internal")
    d_tab1 = nc.dram_tensor("tab1", [NP, ROW1], BF16, kind="Internal",
                            addr_space=shared)
    d_t2loc = nc.dram_tensor("t2loc", [Nlp, ROW2], BF16, kind="Internal")
    d_tab2 = nc.dram_tensor("tab2", [NP, ROW2], BF16, kind="Internal",
                            addr_space=shared)
    d_er1 = nc.dram_tensor("er1", [NB, P, 8], BF16, kind="Internal")
    d_er2 = nc.dram_tensor("er2", [NB, P, 8], BF16, kind="Internal")

    with tile.TileContext(nc) as tc:
        with (
            tc.tile_pool(name="const", bufs=1) as cpool,
            tc.tile_pool(name="work", bufs=3) as wpool,
            tc.tile_pool(name="gath", bufs=3) as gpool,
        ):
            ident = cpool.tile([P, P], BF16)
            make_identity(nc, ident[:])
            rhs1 = cpool.tile([P, F1 + 16], BF16)
            nc.sync.dma_start(rhs1[:], d_RHS1[:, :])
            rhs2 = cpool.tile([P, 2, F2 + 16], BF16)
            nc.sync.dma_start(rhs2[:],
                              d_RHS2[:, :, :].rearrange("q p x -> p q x"))
            b1m = cpool.tile([P, F1], F32)
            nc.sync.dma_start(b1m[:], d_B1M[:, :])
            b2m = cpool.tile([P, OUTW], F32)
            nc.sync.dma_start(b2m[:], d_B2M[:, :])
            hTs = cpool.tile([P, 2, Nlp], BF16)   # persistent h^T (layer 1)

            # ---------------- table shard build (both layers) ----------
            def table_shard(layer, tps):
                FE = (F1 if layer == 1 else F2) + 8
                d_loc = d_t1loc if layer == 1 else d_t2loc
                d_er = d_er1 if layer == 1 else d_er2
                for g in range(NG):
                    if layer == 1:
                        xg = wpool.tile([P, GRP, P], BF16, tag="xg")
                        nc.sync.dma_start(
                            xg[:], d_xTl[:, g * GRP * P:(g + 1) * GRP * P]
                            .rearrange("p (t c) -> p t c", t=GRP))
                    feg = wpool.tile([P, GRP, FE + 8], BF16, tag="feg")
                    for t in range(GRP):
                        ps = tps.tile([P, FE + 8], F32, tag="tbl")
                        if layer == 1:
                            nc.tensor.matmul(ps[:], lhsT=xg[:, t, :],
                                             rhs=rhs1[:], start=True,
                                             stop=True)
                        else:
                            tr = slice((g * GRP + t) * P,
                                       (g * GRP + t + 1) * P)
                            nc.tensor.matmul(ps[:], lhsT=hTs[:, 0, tr],
                                             rhs=rhs2[:, 0, :], start=True,
                                             stop=False)
                            nc.tensor.matmul(ps[:], lhsT=hTs[:, 1, tr],
                                             rhs=rhs2[:, 1, :], start=False,
                                             stop=True)
                        (nc.vector.tensor_copy if t % 2 == 0
                         else nc.scalar.copy)(feg[:, t, :], ps[:])
                    r0 = g * GRP * P
                    nc.sync.dma_start(
                        d_loc[r0:r0 + GRP * P, 0:FE]
                        .rearrange("(t p) x -> p t x", p=P),
                        feg[:, :, 0:FE])
                    nc.sync.dma_start(
                        d_er[g * GRP:(g + 1) * GRP, :, :]
                        .rearrange("t p x -> p t x"),
                        feg[:, :, FE:FE + 8])

            # tab1 is built replicated on every core (the collective cost
            # model makes a 38MB AllGather pricier than redundant matmuls)
            GRP1 = 14
            NT1 = NP // P
            with tc.tile_pool(name="t1ps", bufs=4, space="PSUM") as t1ps:
                for g in range(NT1 // GRP1):
                    xg1 = wpool.tile([P, GRP1, P], BF16, tag="xg1")
                    nc.sync.dma_start(
                        xg1[:], d_XT[:, g * GRP1 * P:(g + 1) * GRP1 * P]
                        .rearrange("p (t c) -> p t c", t=GRP1))
                    feg1 = wpool.tile([P, GRP1, F1 + 8], BF16, tag="feg1")
                    for t in range(GRP1):
                        ps = t1ps.tile([P, F1 + 8], F32, tag="tbl1")
                        nc.tensor.matmul(ps[:], lhsT=xg1[:, t, :],
                                         rhs=rhs1[:, 0:F1 + 8], start=True,
                                         stop=True)
                        (nc.vector.tensor_copy if t % 2 == 0
                         else nc.scalar.copy)(feg1[:, t, :], ps[:])
                    r0 = g * GRP1 * P
                    nc.sync.dma_start(
                        d_tab1[r0:r0 + GRP1 * P, 0:F1 + 8]
                        .rearrange("(t p) x -> p t x", p=P),
                        feg1[:, :, :])
                # er1 for local rows (from xTl)
                for g in range(NG):
                    xlg = wpool.tile([P, GRP, P], BF16, tag="xlg")
                    nc.sync.dma_start(
                        xlg[:], d_xTl[:, g * GRP * P:(g + 1) * GRP * P]
                        .rearrange("p (t c) -> p t c", t=GRP))
                    erg1 = wpool.tile([P, GRP, 8], BF16, tag="erg1")
                    for t in range(GRP):
                        ps = t1ps.tile([P, 8], F32, tag="er1")
                        nc.tensor.matmul(ps[:], lhsT=xlg[:, t, :],
                                         rhs=rhs1[:, F1 + 8:F1 + 16],
                                         start=True, stop=True)
                        nc.vector.tensor_copy(erg1[:, t, :], ps[:])
                    nc.sync.dma_start(
                        d_er1[g * GRP:(g + 1) * GRP, :, :]
                        .rearrange("t p x -> p t x"),
                        erg1[:])

            # ---------------- edge phase (shared) ----------------
            def edge_phase(layer, pspool, tps, finalize):
                F = F1 if layer == 1 else F2
                ROW = ROW1 if layer == 1 else ROW2
                DW = 32 if layer == 1 else 16
                tab = d_tab1 if layer == 1 else d_tab2
                d_er = d_er1 if layer == 1 else d_er2
                pending = None
                for g in range(NG):
                    ixg = wpool.tile([P, GRP, S16], I16, tag="ixg")
                    nc.sync.dma_start(
                        ixg[:], d_IDX[:, g * GRP * S16:(g + 1) * GRP * S16]
                        .rearrange("p (t s) -> p t s", t=GRP))
                    erg = wpool.tile([P, GRP, 8], BF16, tag="erg")
                    nc.sync.dma_start(
                        erg[:], d_er[g * GRP:(g + 1) * GRP, :, :]
                        .rearrange("t p x -> p t x"))
                    for j in range(GRP):
                        b = g * GRP + j
                        oz = gpool.tile([P, 2, NCT, P], FP8, tag="oz")
                        nc.sync.dma_start(oz[:], d_OHZ[:, b, :, :, :])
                        G = gpool.tile([P, NCT, ROW], BF16, tag=f"G{layer}")
                        PIECE = 1024  # 64-desc/lane packet cap
                        for s0 in range(0, SLO, PIECE):
                            n = min(PIECE, SLO - s0)
                            nc.gpsimd.dma_gather(
                                out_ap=G[:, s0 // P:(s0 + n) // P, :],
                                in_ap=tab[:, :],
                                idxs_ap=ixg[:, j, s0 // 16:(s0 + n) // 16],
                                num_idxs=n, num_idxs_reg=n, elem_size=ROW)
                        for s0 in range(SLO, SLO + SHI, PIECE):
                            n = min(PIECE, SLO + SHI - s0)
                            nc.gpsimd.dma_gather(
                                out_ap=G[:, s0 // P:(s0 + n) // P, :],
                                in_ap=tab[cfg.HALF:NP, :],
                                idxs_ap=ixg[:, j, s0 // 16:(s0 + n) // 16],
                                num_idxs=n, num_idxs_reg=n, elem_size=ROW)
                        erps = tps.tile([P, NCT, 8], F32, tag="erps")
                        for c in range(NCT):
                            nc.tensor.matmul(erps[:, c, :],
                                             lhsT=oz[:, 1, c, :],
                                             rhs=erg[:, j, :],
                                             start=True, stop=True)
                        ea = wpool.tile([P, NCT, 8], F32, tag="ea")
                        nc.vector.tensor_tensor(out=ea[:],
                                                in0=G[:, :, F:F + 8],
                                                in1=erps[:], op=OP.add)
                        es = wpool.tile([P, NCT, 8], F32, tag="es")
                        nc.vector.tensor_scalar(es[:], ea[:], cfg.NEG, None,
                                                op0=OP.mult)
                        nc.vector.tensor_tensor(out=es[:], in0=ea[:],
                                                in1=es[:], op=OP.max)
                        nc.scalar.activation(G[:, :, F:F + 8], es[:], AF.Exp)
                        # interleave the exp-multiply and the scatter matmuls
                        # in ~NCT/3-chunk groups so the PE queue never stalls
                        # behind the whole multiply
                        ps = pspool.tile([P, F + 8], F32, tag="eps")
                        import os as _os
                        _ns = int(_os.environ.get("GAT_SPLIT", "3"))
                        splits = [NCT * i // _ns for i in range(_ns + 1)]
                        for c0, c1 in zip(splits[:-1], splits[1:]):
                            nc.vector.tensor_tensor(
                                out=G[:, c0:c1, 0:F].rearrange(
                                    "p c (d h) -> p c d h", h=8),
                                in0=G[:, c0:c1, 0:F].rearrange(
                                    "p c (d h) -> p c d h", h=8),
                                in1=G[:, c0:c1, F:F + 8].rearrange(
                                    "p c (one h) -> p c one h", one=1)
                                    .to_broadcast([P, c1 - c0, DW, 8]),
                                op=OP.mult)
                            for c in range(c0, c1):
                                nc.tensor.matmul(ps[:], lhsT=oz[:, 0, c, :],
                                                 rhs=G[:, c, 0:F + 8],
                                                 start=(c == 0),
                                                 stop=(c == NCT - 1))
                        esum = wpool.tile([P, 8], F32, tag="esum")
                        # layer 2 folds the head-mean 1/8 into the reciprocal
                        if layer == 2:
                            nc.vector.tensor_scalar(esum[:], ps[:, F:F + 8],
                                                    1e-30, 8.0, op0=OP.max,
                                                    op1=OP.mult)
                        else:
                            nc.vector.tensor_scalar(esum[:], ps[:, F:F + 8],
                                                    1e-30, None, op0=OP.max)
                        inv = wpool.tile([P, 8], F32, tag="inv")
                        nc.vector.reciprocal(inv[:], esum[:])
                        if not _os.environ.get("GAT_DELAY"):
                            finalize(b, ps, inv)
                        else:
                            if pending is not None:
                                finalize(*pending)
                            pending = (b, ps, inv)
                if pending is not None:
                    finalize(*pending)

            # ---------------- layer-1 edges -> hTs ----------------
            with tc.tile_pool(name="e1ps", bufs=3, space="PSUM") as e1ps, \
                 tc.tile_pool(name="tps", bufs=2, space="PSUM") as tps:
                def fin1(b, ps, inv):
                    z = wpool.tile([P, F1], F32, tag="z")
                    nc.vector.tensor_tensor(
                        out=z[:].rearrange("p (d h) -> p d h", h=8),
                        in0=ps[:, 0:F1].rearrange("p (d h) -> p d h", h=8),
                        in1=inv[:].rearrange("p (one h) -> p one h", one=1)
                            .to_broadcast([P, 32, 8]),
                        op=OP.mult)
                    zb = wpool.tile([P, F1], F32, tag="zb")
                    nc.vector.tensor_tensor(out=zb[:], in0=z[:], in1=b1m[:],
                                            op=OP.add)
                    zm = wpool.tile([P, F1], F32, tag="zm")
                    nc.vector.tensor_scalar(zm[:], zb[:], 0.0, None,
                                            op0=OP.min)
                    eb = wpool.tile([P, F1], BF16, tag="eb")
                    nc.scalar.activation(eb[:], zm[:], AF.Exp)
                    rb = wpool.tile([P, F1], BF16, tag="rb")
                    nc.scalar.activation(rb[:], zb[:], AF.Relu)
                    hs = wpool.tile([P, F1], BF16, tag="hs")
                    nc.vector.tensor_tensor(out=hs[:], in0=eb[:], in1=rb[:],
                                            op=OP.add)
                    hm = wpool.tile([P, F1], BF16, tag="hm")
                    nc.vector.tensor_scalar(hm[:], hs[:], -1.0, None,
                                            op0=OP.add)
                    for q in range(2):
                        pst = tps.tile([P, P], BF16, tag="pst")
                        nc.tensor.transpose(pst[:], hm[:, q * P:(q + 1) * P],
                                            ident[:])
                        (nc.vector.tensor_copy if q == 0
                         else nc.scalar.copy)(
                            hTs[:, q, b * P:(b + 1) * P], pst[:])

                edge_phase(1, e1ps, tps, fin1)

            # ---------------- layer-2 table + AG ----------------
            with tc.tile_pool(name="t2ps", bufs=3, space="PSUM") as t2ps:
                table_shard(2, t2ps)
            if not skip_cc:
                nc.gpsimd.collective_compute(
                    "AllGather", OP.bypass,
                    replica_groups=[list(range(C))],
                    ins=[d_t2loc[:, :]], outs=[d_tab2[:, :]])

            # ---------------- layer-2 edges -> out ----------------
            with tc.tile_pool(name="e2ps", bufs=3, space="PSUM") as e2ps, \
                 tc.tile_pool(name="tps2", bufs=2, space="PSUM") as tps2:
                ogbox = [None]

                def fin2(b, ps, inv):
                    if b % GRP == 0:
                        ogbox[0] = wpool.tile([P, GRP, OUTW], F32, tag="og",
                                              name="og")
                    og = ogbox[0]
                    w_ = wpool.tile([P, F2], F32, tag="w_")
                    nc.vector.tensor_tensor(
                        out=w_[:].rearrange("p (o h) -> p o h", h=8),
                        in0=ps[:, 0:F2].rearrange("p (o h) -> p o h", h=8),
                        in1=inv[:].rearrange("p (one h) -> p one h", one=1)
                            .to_broadcast([P, OUTW, 8]),
                        op=OP.mult)
                    ws = wpool.tile([P, OUTW], F32, tag="ws")
                    nc.vector.tensor_reduce(
                        ws[:], w_[:].rearrange("p (o h) -> p o h", h=8),
                        axis=mybir.AxisListType.X, op=OP.add)
                    nc.vector.tensor_tensor(out=og[:, b % GRP, :], in0=ws[:],
                                            in1=b2m[:], op=OP.add)
                    if b % GRP == GRP - 1:
                        r0 = (b - GRP + 1) * P
                        nc.sync.dma_start(
                            d_out[r0:r0 + GRP * P, :]
                            .rearrange("(t p) x -> p t x", p=P),
                            og[:, :, :])

                edge_phase(2, e2ps, tps2, fin2)

            if dbg:
                for nm, src_t in [("dbg_t1loc", d_t1loc), ("dbg_er1", d_er1),
                                  ("dbg_tab1", d_tab1), ("dbg_t2loc", d_t2loc),
                                  ("dbg_er2", d_er2), ("dbg_tab2", d_tab2)]:
                    dd = nc.dram_tensor(nm, list(src_t.shape), BF16,
                                        kind="ExternalOutput")
                    sl = tuple(slice(None) for _ in src_t.shape)
                    nc.sync.dma_start(dd[sl], src_t[sl])

    nc.compile()
    return nc


# ----------------------------------------------------------------------------
_CACHE = {}


def get_built(src, dst, C=8, cfg=None):
    key = (hash(src.tobytes()), hash(dst.tobytes()), C)
    if key not in _CACHE:
        if cfg is None:
            cfg = GATCfg(C=C)
        idx = prep_indices(src, dst, cfg)
        nc = build_module(cfg)
        _CACHE[key] = (cfg, idx, nc)
    return _CACHE[key]


_EXECC = {}


def _get_exec(key, nc, n_cores):
    """Persistent jit(shard_map(bass_exec)) so repeated kernel() calls skip
    retracing/recompiling."""
    if key in _EXECC:
        return _EXECC[key]
    import jax
    from jax.experimental.shard_map import shard_map
    from jax.sharding import Mesh, NamedSharding, PartitionSpec
    from concourse import bass2jax
    bass2jax.install_neuronx_cc_hook()
    partition_name = (nc.partition_id_tensor.name
                      if nc.partition_id_tensor else None)
    in_names, out_names, out_avals, zero_shapes = [], [], [], []
    for alloc in nc.m.functions[0].allocations:
        if not isinstance(alloc, mybir.MemoryLocationSet):
            continue
        name = alloc.memorylocations[0].name
        if alloc.kind == "ExternalInput":
            if name != partition_name:
                in_names.append(name)
        elif alloc.kind == "ExternalOutput":
            out_names.append(name)
            shape = tuple(alloc.tensor_shape)
            dtype = mybir.dt.np(alloc.dtype)
            out_avals.append(jax.core.ShapedArray(shape, dtype))
            zero_shapes.append((shape, dtype))
    n_params = len(in_names)
    in_names_all = list(in_names) + out_names + (
        [partition_name] if partition_name else [])

    def _body(*args):
        ops = list(args)
        if partition_name:
            ops.append(bass2jax.partition_id_tensor())
        outs = bass2jax._bass_exec_p.bind(
            *ops, out_avals=tuple(out_avals), in_names=tuple(in_names_all),
            out_names=tuple(out_names), lowering_input_output_aliases=(),
            sim_require_finite=True, sim_require_nnan=True, nc=nc)
        return tuple(outs)

    devices = jax.devices()[:n_cores]
    mesh = Mesh(np.asarray(devices), ("core",))
    nout = len(out_names)
    f = jax.jit(shard_map(
        _body, mesh=mesh,
        in_specs=(PartitionSpec("core"),) * (n_params + nout),
        out_specs=(PartitionSpec("core"),) * nout, check_rep=False),
        keep_unused=True)
    sh = NamedSharding(mesh, PartitionSpec("core"))
    ent = dict(f=f, in_names=in_names, out_names=out_names,
               zero_shapes=zero_shapes, sh=sh, argcache=None)
    _EXECC[key] = ent
    return ent


def kernel(**inputs) -> np.ndarray:
    import jax
    src = np.asarray(inputs["src"], np.int32)
    dst = np.asarray(inputs["dst"], np.int32)
    x = np.asarray(inputs["x"])
    base = GATCfg(N=int(x.shape[0]), C=8, IN=int(x.shape[1]))
    cfg, idx, nc = get_built(src, dst, C=8, cfg=base)
    in_maps = host_inputs(inputs, cfg, idx)
    key = (hash(src.tobytes()), hash(dst.tobytes()), cfg.C)
    ent = _get_exec(key, nc, cfg.C)
    C = cfg.C
    concat_in = [np.ascontiguousarray(
        np.concatenate([in_maps[c][nm] for c in range(C)], axis=0))
        for nm in ent["in_names"]]
    hashes = tuple(hash(a.tobytes()) for a in concat_in)
    if ent["argcache"] is None or ent["argcache"][0] != hashes:
        zeros = [np.zeros((C * sh0[0], *sh0[1:]), dt)
                 for sh0, dt in ent["zero_shapes"]]
        args = [jax.device_put(a, ent["sh"]) for a in concat_in + zeros]
        ent["argcache"] = (hashes, args)
    args = ent["argcache"][1]
    outs = ent["f"](*args)
    jax.block_until_ready(outs)
    oi = ent["out_names"].index("out")
    out = np.asarray(outs[oi]).reshape(C, cfg.Nlp, cfg.OUT)
    return out.reshape(-1, cfg.OUT)[:cfg.N].astype(np.float32)


# revision 37
# speedup vs baseline: 1.2535x; 1.1796x over previous
"""2-layer GAT (graph attention) Bass/Tile kernel for Trainium2, 8-core SPMD.

Sharding: nodes partitioned into 6272-row grid-aligned slices (8 x 6272 =
50176 = the 128-padded node grid), edges owned by the dst core, sorted by
dst 128-block, lo/hi-split per block for int16 gather indexing.

Per core and per layer:
  - build the LOCAL feature-table shard ([feat | el | er] columns in one
    matmul per 128-node tile, feature columns interleaved (d, h) so the
    per-head multiplies have packed last dims), AllGather the shard into the
    full [50176, ROW] bf16 table.
  - edge phase per 128-dst block: two dma_gathers (lo/hi) fetch src rows
    [feat | el]; host-precomputed fp8 one-hot matrices give (a) er broadcast
    to edge slots and (b) the dst scatter, both as PE matmuls; attention
    softmax skips max-subtraction (|e| is O(1)); exp and elu run on the
    scalar engine.
All DMAs are batched (few, large, strided) to keep the serialized HWDGE
descriptor queue off the critical path.
"""

import numpy as np
import ml_dtypes

import concourse.bacc as bacc
import concourse.bass as bass
import concourse.mybir as mybir
import concourse.tile as tile
from concourse.masks import make_identity

F32 = mybir.dt.float32
BF16 = mybir.dt.bfloat16
I16 = mybir.dt.int16
FP8 = mybir.dt.float8e4
AF = mybir.ActivationFunctionType
OP = mybir.AluOpType

P = 128
NPBF = ml_dtypes.bfloat16
NPF8 = ml_dtypes.float8_e4m3
ONE8 = np.float32(1.0).astype(NPF8).view(np.uint8)  # fp8 bit pattern of 1.0


class GATCfg:
    def __init__(self, N=50000, C=8, IN=128, HID=32, HEADS=8, OUT=16, NEG=0.2):
        self.N, self.C, self.IN = N, C, IN
        self.HID, self.HEADS, self.OUT, self.NEG = HID, HEADS, OUT, NEG
        self.NP = ((N + C * P - 1) // (C * P)) * (C * P)   # 50176
        self.Nlp = self.NP // C                             # 6272
        self.NB = self.Nlp // P                             # 49
        self.HALF = self.NP // 2                            # tuned by prep
        self.F1 = HEADS * HID                               # 256
        self.F2 = HEADS * OUT                               # 128
        self.ROW1 = 384   # bf16 elems: 768B rows (256B multiple)
        self.ROW2 = 256   # 512B rows
        self.SLO = 0
        self.SHI = 0
        self.GRP = 7      # blocks per load/store group (NB = 7*7)

    @property
    def NCT(self):
        return (self.SLO + self.SHI) // P

    @property
    def NLO(self):
        return self.SLO // P


def prep_indices(src, dst, cfg):
    """Host index preprocessing: per-core per-block slot assignment, wrapped
    int16 gather indices, and fp8 one-hot (scatter + er-broadcast) tensors.
    Shared by both layers (same edges, same node grid)."""
    C, Nlp, NB = cfg.C, cfg.Nlp, cfg.NB
    src = np.asarray(src).astype(np.int64)
    dst = np.asarray(dst).astype(np.int64)
    core = dst // Nlp
    dloc = dst - core * Nlp
    blk = dloc // P
    dblk = dloc - blk * P

    # tune HALF (int16 split point) to minimize gather calls, then padding:
    # calls/block = ceil(SLO/1024) + ceil(SHI/1024) with SLO/SHI the
    # 128-rounded per-block max lo/hi counts.  HALF must keep both index
    # halves < 32768.
    bkey = core * NB + blk
    border = np.argsort(bkey, kind="stable")
    bcnt = np.bincount(bkey, minlength=C * NB)
    bstart = np.zeros(C * NB + 1, np.int64)
    np.cumsum(bcnt, out=bstart[1:])
    ssrc = src[border]
    srt = np.zeros_like(ssrc)
    for k in range(C * NB):
        seg = np.sort(ssrc[bstart[k]:bstart[k + 1]])
        srt[bstart[k]:bstart[k + 1]] = seg
    best = None
    for half in range(cfg.NP - 32768, 32768 + 128, 128):
        lo_max = 0
        hi_max = 0
        for k in range(C * NB):
            seg = srt[bstart[k]:bstart[k + 1]]
            nlo = int(np.searchsorted(seg, half))
            lo_max = max(lo_max, nlo)
            hi_max = max(hi_max, seg.size - nlo)
        SLO = max(P, ((lo_max + P - 1) // P) * P)
        SHI = ((hi_max + P - 1) // P) * P
        calls = -(-SLO // 1024) + -(-SHI // 1024)
        score = (calls, SLO + SHI)
        if best is None or score < best[0]:
            best = (score, half, SLO, SHI)
    _, HALF, SLO, SHI = best
    cfg.HALF, cfg.SLO, cfg.SHI = HALF, SLO, SHI
    is_hi = src >= HALF

    # order edges by (core, block, is_hi); compute slot-in-group
    key = (core * NB + blk) * 2 + is_hi
    order = np.argsort(key, kind="stable")
    kord = key[order]
    counts = np.bincount(kord, minlength=C * NB * 2)
    starts = np.zeros(C * NB * 2 + 1, np.int64)
    np.cumsum(counts, out=starts[1:])
    pos_in_grp = np.arange(len(order)) - starts[kord]
    S = SLO + SHI
    NCT = S // P
    S16 = S // 16

    e = order
    ecore = core[e]
    eblk = blk[e]
    edb = dblk[e]
    ehi = is_hi[e]
    eslot = pos_in_grp + np.where(ehi, SLO, 0)
    eidx = np.where(ehi, src[e] - HALF, src[e])

    # slot-ordered index values [C, NB, S]
    vals = np.zeros((C, NB, S), np.int64)
    vals[ecore, eblk, eslot] = eidx
    # wrap16: [C, NB, S] -> [C, 128, NB*S16] (16-wrapped, replicated 8x)
    w = vals.reshape(C, NB, S16, 16).transpose(0, 3, 1, 2)  # [C,16,NB,S16]
    w = w.reshape(C, 16, NB * S16).astype(np.int16)
    idxw = np.tile(w, (1, 8, 1))                            # [C,128,NB*S16]

    # fp8 one-hots [C, 128, NB, 2, NCT, 128]
    ohz = np.zeros((C, P, NB, 2, NCT, P), np.uint8)
    ech = eslot // P
    epp = eslot - ech * P
    ohz[ecore, epp, eblk, 0, ech, edb] = ONE8
    ohz[ecore, edb, eblk, 1, ech, epp] = ONE8
    return {"idx": idxw, "ohz": ohz.view(NPF8)}


def _perm_dh(H, D):
    """Column permutation (h, d) -> (d, h): newcol[d*H+h] = oldcol[h*D+d]."""
    pm = np.zeros(H * D, np.int64)
    for h in range(H):
        for d in range(D):
            pm[d * H + h] = h * D + d
    return pm


def host_inputs(inputs, cfg, idx):
    N, C, IN = cfg.N, cfg.C, cfg.IN
    H, D1, D2 = cfg.HEADS, cfg.HID, cfg.OUT
    F1, F2, Nlp = cfg.F1, cfg.F2, cfg.Nlp
    x = np.asarray(inputs["x"], np.float32)
    W1 = np.asarray(inputs["W1"], np.float32)
    W2 = np.asarray(inputs["W2"], np.float32)
    al1 = np.asarray(inputs["al1"], np.float32)
    ar1 = np.asarray(inputs["ar1"], np.float32)
    al2 = np.asarray(inputs["al2"], np.float32)
    ar2 = np.asarray(inputs["ar2"], np.float32)
    b1 = np.asarray(inputs["b1"], np.float32)
    b2 = np.asarray(inputs["b2"], np.float32)

    p1 = _perm_dh(H, D1)
    p2 = _perm_dh(H, D2)
    Wel1 = np.einsum("ihd,hd->ih", W1.reshape(IN, H, D1), al1)
    Wer1 = np.einsum("ihd,hd->ih", W1.reshape(IN, H, D1), ar1)
    RHS1 = np.concatenate([W1[:, p1], Wel1, Wer1], axis=1).astype(NPBF)

    W2p = W2[p1][:, p2]                       # rows (d,h), cols (o,h2)
    Wel2 = np.einsum("rhd,hd->rh", W2.reshape(F1, H, D2), al2)[p1]
    Wer2 = np.einsum("rhd,hd->rh", W2.reshape(F1, H, D2), ar2)[p1]
    RHS2 = np.concatenate([W2p, Wel2, Wer2], axis=1)       # [256, 144]
    RHS2 = RHS2.reshape(2, P, F2 + 16).astype(NPBF)

    B1M = np.broadcast_to(b1[p1][None, :], (P, F1)).copy()
    b2m = b2.reshape(H, D2).mean(axis=0)
    B2M = np.broadcast_to(b2m[None, :], (P, D2)).copy()

    xT = np.zeros((IN, cfg.NP), np.float32)
    xT[:, :N] = x.T
    XT = xT.astype(NPBF)

    in_maps = []
    for c in range(C):
        in_maps.append({
            "xTl": XT[:, c * Nlp:(c + 1) * Nlp].copy(),
            "RHS1": RHS1, "RHS2": RHS2, "B1M": B1M, "B2M": B2M,
            "IDX": idx["idx"][c], "OHZ": idx["ohz"][c],
        })
    return in_maps


def build_module(cfg, dbg=False, skip_cc=False):
    nc = bacc.Bacc("TRN2", target_bir_lowering=False, debug=False,
                   num_devices=cfg.C)
    C, NB, Nlp, NP = cfg.C, cfg.NB, cfg.Nlp, cfg.NP
    F1, F2, ROW1, ROW2 = cfg.F1, cfg.F2, cfg.ROW1, cfg.ROW2
    SLO, SHI, NCT, NLO = cfg.SLO, cfg.SHI, cfg.NCT, cfg.NLO
    GRP = cfg.GRP
    NG = NB // GRP
    S16 = (SLO + SHI) // 16
    L16 = SLO // 16
    H16 = SHI // 16
    OUTW = cfg.OUT

    d_xTl = nc.dram_tensor("xTl", [cfg.IN, Nlp], BF16, kind="ExternalInput")
    d_RHS1 = nc.dram_tensor("RHS1", [cfg.IN, F1 + 16], BF16,
                            kind="ExternalInput")
    d_RHS2 = nc.dram_tensor("RHS2", [2, P, F2 + 16], BF16,
                            kind="ExternalInput")
    d_B1M = nc.dram_tensor("B1M", [P, F1], F32, kind="ExternalInput")
    d_B2M = nc.dram_tensor("B2M", [P, OUTW], F32, kind="ExternalInput")
    d_IDX = nc.dram_tensor("IDX", [P, NB * S16], I16, kind="ExternalInput")
    d_OHZ = nc.dram_tensor("OHZ", [P, NB, 2, NCT, P], FP8,
                           kind="ExternalInput")
    d_out = nc.dram_tensor("out", [Nlp, OUTW], F32, kind="ExternalOutput")

    shared = "Shared" if C > 4 else "Local"
    d_t1loc = nc.dram_tensor("t1loc", [Nlp, ROW1], BF16, kind="Internal")
    d_tab1 = nc.dram_tensor("tab1", [NP, ROW1], BF16, kind="Internal",
                            addr_space=shared)
    d_t2loc = nc.dram_tensor("t2loc", [Nlp, ROW2], BF16, kind="Internal")
    d_tab2 = nc.dram_tensor("tab2", [NP, ROW2], BF16, kind="Internal",
                            addr_space=shared)
    d_er1 = nc.dram_tensor("er1", [NB, P, 8], BF16, kind="Internal")
    d_er2 = nc.dram_tensor("er2", [NB, P, 8], BF16, kind="Internal")

    with tile.TileContext(nc) as tc:
        with (
            tc.tile_pool(name="const", bufs=1) as cpool,
            tc.tile_pool(name="work", bufs=3) as wpool,
            tc.tile_pool(name="gath", bufs=3) as gpool,
        ):
            ident = cpool.tile([P, P], BF16)
            make_identity(nc, ident[:])
            rhs1 = cpool.tile([P, F1 + 16], BF16)
            nc.sync.dma_start(rhs1[:], d_RHS1[:, :])
            rhs2 = cpool.tile([P, 2, F2 + 16], BF16)
            nc.sync.dma_start(rhs2[:],
                              d_RHS2[:, :, :].rearrange("q p x -> p q x"))
            b1m = cpool.tile([P, F1], F32)
            nc.sync.dma_start(b1m[:], d_B1M[:, :])
            b2m = cpool.tile([P, OUTW], F32)
            nc.sync.dma_start(b2m[:], d_B2M[:, :])
            hTs = cpool.tile([P, 2, Nlp], BF16)   # persistent h^T (layer 1)

            # ---------------- table shard build (both layers) ----------
            def table_shard(layer, tps):
                FE = (F1 if layer == 1 else F2) + 8
                d_loc = d_t1loc if layer == 1 else d_t2loc
                d_er = d_er1 if layer == 1 else d_er2
                for g in range(NG):
                    if layer == 1:
                        xg = wpool.tile([P, GRP, P], BF16, tag="xg")
                        nc.sync.dma_start(
                            xg[:], d_xTl[:, g * GRP * P:(g + 1) * GRP * P]
                            .rearrange("p (t c) -> p t c", t=GRP))
                    feg = wpool.tile([P, GRP, FE + 8], BF16, tag="feg")
                    for t in range(GRP):
                        ps = tps.tile([P, FE + 8], F32, tag="tbl")
                        if layer == 1:
                            nc.tensor.matmul(ps[:], lhsT=xg[:, t, :],
                                             rhs=rhs1[:], start=True,
                                             stop=True)
                        else:
                            tr = slice((g * GRP + t) * P,
                                       (g * GRP + t + 1) * P)
                            nc.tensor.matmul(ps[:], lhsT=hTs[:, 0, tr],
                                             rhs=rhs2[:, 0, :], start=True,
                                             stop=False)
                            nc.tensor.matmul(ps[:], lhsT=hTs[:, 1, tr],
                                             rhs=rhs2[:, 1, :], start=False,
                                             stop=True)
                        (nc.vector.tensor_copy if t % 2 == 0
                         else nc.scalar.copy)(feg[:, t, :], ps[:])
                    r0 = g * GRP * P
                    nc.sync.dma_start(
                        d_loc[r0:r0 + GRP * P, 0:FE]
                        .rearrange("(t p) x -> p t x", p=P),
                        feg[:, :, 0:FE])
                    nc.sync.dma_start(
                        d_er[g * GRP:(g + 1) * GRP, :, :]
                        .rearrange("t p x -> p t x"),
                        feg[:, :, FE:FE + 8])

            with tc.tile_pool(name="t1ps", bufs=4, space="PSUM") as t1ps:
                table_shard(1, t1ps)
            if not skip_cc:
                nc.gpsimd.collective_compute(
                    "AllGather", OP.bypass,
                    replica_groups=[list(range(C))],
                    ins=[d_t1loc[:, :]], outs=[d_tab1[:, :]])

            # ---------------- edge phase (shared) ----------------
            def edge_phase(layer, pspool, tps, finalize, cc_hook=None):
                F = F1 if layer == 1 else F2
                ROW = ROW1 if layer == 1 else ROW2
                DW = 32 if layer == 1 else 16
                tab = d_tab1 if layer == 1 else d_tab2
                d_er = d_er1 if layer == 1 else d_er2
                pending = None
                for g in range(NG):
                    ixg = wpool.tile([P, GRP, S16], I16, tag=f"ixg{layer}")
                    nc.sync.dma_start(
                        ixg[:], d_IDX[:, g * GRP * S16:(g + 1) * GRP * S16]
                        .rearrange("p (t s) -> p t s", t=GRP))
                    erg = wpool.tile([P, GRP, 8], BF16, tag=f"erg{layer}")
                    nc.sync.dma_start(
                        erg[:], d_er[g * GRP:(g + 1) * GRP, :, :]
                        .rearrange("t p x -> p t x"))
                    for j in range(GRP):
                        b = g * GRP + j
                        oz = gpool.tile([P, 2, NCT, P], FP8, tag=f"oz{layer}")
                        nc.sync.dma_start(oz[:], d_OHZ[:, b, :, :, :])
                        if cc_hook is not None and b == 0:
                            cc_hook()
                        G = gpool.tile([P, NCT, ROW], BF16, tag=f"G{layer}")
                        PIECE = 1024  # 64-desc/lane packet cap
                        for s0 in range(0, SLO, PIECE):
                            n = min(PIECE, SLO - s0)
                            nc.gpsimd.dma_gather(
                                out_ap=G[:, s0 // P:(s0 + n) // P, :],
                                in_ap=tab[:, :],
                                idxs_ap=ixg[:, j, s0 // 16:(s0 + n) // 16],
                                num_idxs=n, num_idxs_reg=n, elem_size=ROW)
                        for s0 in range(SLO, SLO + SHI, PIECE):
                            n = min(PIECE, SLO + SHI - s0)
                            nc.gpsimd.dma_gather(
                                out_ap=G[:, s0 // P:(s0 + n) // P, :],
                                in_ap=tab[cfg.HALF:NP, :],
                                idxs_ap=ixg[:, j, s0 // 16:(s0 + n) // 16],
                                num_idxs=n, num_idxs_reg=n, elem_size=ROW)
                        erps = tps.tile([P, NCT, 8], F32, tag="erps")
                        for c in range(NCT):
                            nc.tensor.matmul(erps[:, c, :],
                                             lhsT=oz[:, 1, c, :],
                                             rhs=erg[:, j, :],
                                             start=True, stop=True)
                        ea = wpool.tile([P, NCT, 8], F32, tag=f"ea{layer}")
                        nc.vector.tensor_tensor(out=ea[:],
                                                in0=G[:, :, F:F + 8],
                                                in1=erps[:], op=OP.add)
                        es = wpool.tile([P, NCT, 8], F32, tag=f"es{layer}")
                        nc.vector.tensor_scalar(es[:], ea[:], cfg.NEG, None,
                                                op0=OP.mult)
                        nc.vector.tensor_tensor(out=es[:], in0=ea[:],
                                                in1=es[:], op=OP.max)
                        nc.scalar.activation(G[:, :, F:F + 8], es[:], AF.Exp)
                        # interleave the exp-multiply and the scatter matmuls
                        # in ~NCT/3-chunk groups so the PE queue never stalls
                        # behind the whole multiply
                        ps = pspool.tile([P, F + 8], F32, tag="eps")
                        import os as _os
                        _ns = int(_os.environ.get("GAT_SPLIT", "3"))
                        splits = [NCT * i // _ns for i in range(_ns + 1)]
                        for c0, c1 in zip(splits[:-1], splits[1:]):
                            nc.vector.tensor_tensor(
                                out=G[:, c0:c1, 0:F].rearrange(
                                    "p c (d h) -> p c d h", h=8),
                                in0=G[:, c0:c1, 0:F].rearrange(
                                    "p c (d h) -> p c d h", h=8),
                                in1=G[:, c0:c1, F:F + 8].rearrange(
                                    "p c (one h) -> p c one h", one=1)
                                    .to_broadcast([P, c1 - c0, DW, 8]),
                                op=OP.mult)
                            for c in range(c0, c1):
                                nc.tensor.matmul(ps[:], lhsT=oz[:, 0, c, :],
                                                 rhs=G[:, c, 0:F + 8],
                                                 start=(c == 0),
                                                 stop=(c == NCT - 1))
                        esum = wpool.tile([P, 8], F32, tag=f"esum{layer}")
                        # layer 2 folds the head-mean 1/8 into the reciprocal
                        if layer == 2:
                            nc.vector.tensor_scalar(esum[:], ps[:, F:F + 8],
                                                    1e-30, 8.0, op0=OP.max,
                                                    op1=OP.mult)
                        else:
                            nc.vector.tensor_scalar(esum[:], ps[:, F:F + 8],
                                                    1e-30, None, op0=OP.max)
                        inv = wpool.tile([P, 8], F32, tag=f"inv{layer}")
                        nc.vector.reciprocal(inv[:], esum[:])
                        if not _os.environ.get("GAT_DELAY"):
                            finalize(b, ps, inv)
                        else:
                            if pending is not None:
                                finalize(*pending)
                            pending = (b, ps, inv)
                if pending is not None:
                    finalize(*pending)

            # ---------------- layer-1 edges -> hTs ----------------
            with tc.tile_pool(name="e1ps", bufs=3, space="PSUM") as e1ps, \
                 tc.tile_pool(name="tps", bufs=2, space="PSUM") as tps:
                def fin1(b, ps, inv):
                    z = wpool.tile([P, F1], F32, tag="z")
                    nc.vector.tensor_tensor(
                        out=z[:].rearrange("p (d h) -> p d h", h=8),
                        in0=ps[:, 0:F1].rearrange("p (d h) -> p d h", h=8),
                        in1=inv[:].rearrange("p (one h) -> p one h", one=1)
                            .to_broadcast([P, 32, 8]),
                        op=OP.mult)
                    zb = wpool.tile([P, F1], F32, tag="zb")
                    nc.vector.tensor_tensor(out=zb[:], in0=z[:], in1=b1m[:],
                                            op=OP.add)
                    zm = wpool.tile([P, F1], F32, tag="zm")
                    nc.vector.tensor_scalar(zm[:], zb[:], 0.0, None,
                                            op0=OP.min)
                    eb = wpool.tile([P, F1], BF16, tag="eb")
                    nc.scalar.activation(eb[:], zm[:], AF.Exp)
                    rb = wpool.tile([P, F1], BF16, tag="rb")
                    nc.scalar.activation(rb[:], zb[:], AF.Relu)
                    hs = wpool.tile([P, F1], BF16, tag="hs")
                    nc.vector.tensor_tensor(out=hs[:], in0=eb[:], in1=rb[:],
                                            op=OP.add)
                    hm = wpool.tile([P, F1], BF16, tag="hm")
                    nc.vector.tensor_scalar(hm[:], hs[:], -1.0, None,
                                            op0=OP.add)
                    for q in range(2):
                        pst = tps.tile([P, P], BF16, tag="pst")
                        nc.tensor.transpose(pst[:], hm[:, q * P:(q + 1) * P],
                                            ident[:])
                        (nc.vector.tensor_copy if q == 0
                         else nc.scalar.copy)(
                            hTs[:, q, b * P:(b + 1) * P], pst[:])

                edge_phase(1, e1ps, tps, fin1)

            # ---------------- layer-2 table + AG ----------------
            with tc.tile_pool(name="t2ps", bufs=3, space="PSUM") as t2ps:
                table_shard(2, t2ps)

            def cc2():
                if not skip_cc:
                    nc.gpsimd.collective_compute(
                        "AllGather", OP.bypass,
                        replica_groups=[list(range(C))],
                        ins=[d_t2loc[:, :]], outs=[d_tab2[:, :]])

            # ---------------- layer-2 edges -> out ----------------
            with tc.tile_pool(name="e2ps", bufs=3, space="PSUM") as e2ps, \
                 tc.tile_pool(name="tps2", bufs=2, space="PSUM") as tps2:
                ogbox = [None]

                def fin2(b, ps, inv):
                    if b % GRP == 0:
                        ogbox[0] = wpool.tile([P, GRP, OUTW], F32, tag="og",
                                              name="og")
                    og = ogbox[0]
                    w_ = wpool.tile([P, F2], F32, tag="w_")
                    nc.vector.tensor_tensor(
                        out=w_[:].rearrange("p (o h) -> p o h", h=8),
                        in0=ps[:, 0:F2].rearrange("p (o h) -> p o h", h=8),
                        in1=inv[:].rearrange("p (one h) -> p one h", one=1)
                            .to_broadcast([P, OUTW, 8]),
                        op=OP.mult)
                    ws = wpool.tile([P, OUTW], F32, tag="ws")
                    nc.vector.tensor_reduce(
                        ws[:], w_[:].rearrange("p (o h) -> p o h", h=8),
                        axis=mybir.AxisListType.X, op=OP.add)
                    nc.vector.tensor_tensor(out=og[:, b % GRP, :], in0=ws[:],
                                            in1=b2m[:], op=OP.add)
                    if b % GRP == GRP - 1:
                        r0 = (b - GRP + 1) * P
                        nc.sync.dma_start(
                            d_out[r0:r0 + GRP * P, :]
                            .rearrange("(t p) x -> p t x", p=P),
                            og[:, :, :])

                edge_phase(2, e2ps, tps2, fin2, cc_hook=cc2)

            if dbg:
                for nm, src_t in [("dbg_t1loc", d_t1loc), ("dbg_er1", d_er1),
                                  ("dbg_tab1", d_tab1), ("dbg_t2loc", d_t2loc),
                                  ("dbg_er2", d_er2), ("dbg_tab2", d_tab2)]:
                    dd = nc.dram_tensor(nm, list(src_t.shape), BF16,
                                        kind="ExternalOutput")
                    sl = tuple(slice(None) for _ in src_t.shape)
                    nc.sync.dma_start(dd[sl], src_t[sl])

    nc.compile()
    return nc


# ----------------------------------------------------------------------------
_CACHE = {}


def get_built(src, dst, C=8, cfg=None):
    key = (hash(src.tobytes()), hash(dst.tobytes()), C)
    if key not in _CACHE:
        if cfg is None:
            cfg = GATCfg(C=C)
        idx = prep_indices(src, dst, cfg)
        nc = build_module(cfg)
        _CACHE[key] = (cfg, idx, nc)
    return _CACHE[key]


_EXECC = {}


def _get_exec(key, nc, n_cores):
    """Persistent jit(shard_map(bass_exec)) so repeated kernel() calls skip
    retracing/recompiling."""
    if key in _EXECC:
        return _EXECC[key]
    import jax
    from jax.experimental.shard_map import shard_map
    from jax.sharding import Mesh, NamedSharding, PartitionSpec
    from concourse import bass2jax
    bass2jax.install_neuronx_cc_hook()
    partition_name = (nc.partition_id_tensor.name
                      if nc.partition_id_tensor else None)
    in_names, out_names, out_avals, zero_shapes = [], [], [], []
    for alloc in nc.m.functions[0].allocations:
        if not isinstance(alloc, mybir.MemoryLocationSet):
            continue
        name = alloc.memorylocations[0].name
        if alloc.kind == "ExternalInput":
            if name != partition_name:
                in_names.append(name)
        elif alloc.kind == "ExternalOutput":
            out_names.append(name)
            shape = tuple(alloc.tensor_shape)
            dtype = mybir.dt.np(alloc.dtype)
            out_avals.append(jax.core.ShapedArray(shape, dtype))
            zero_shapes.append((shape, dtype))
    n_params = len(in_names)
    in_names_all = list(in_names) + out_names + (
        [partition_name] if partition_name else [])

    def _body(*args):
        ops = list(args)
        if partition_name:
            ops.append(bass2jax.partition_id_tensor())
        outs = bass2jax._bass_exec_p.bind(
            *ops, out_avals=tuple(out_avals), in_names=tuple(in_names_all),
            out_names=tuple(out_names), lowering_input_output_aliases=(),
            sim_require_finite=True, sim_require_nnan=True, nc=nc)
        return tuple(outs)

    devices = jax.devices()[:n_cores]
    mesh = Mesh(np.asarray(devices), ("core",))
    nout = len(out_names)
    f = jax.jit(shard_map(
        _body, mesh=mesh,
        in_specs=(PartitionSpec("core"),) * (n_params + nout),
        out_specs=(PartitionSpec("core"),) * nout, check_rep=False),
        keep_unused=True)
    sh = NamedSharding(mesh, PartitionSpec("core"))
    ent = dict(f=f, in_names=in_names, out_names=out_names,
               zero_shapes=zero_shapes, sh=sh, argcache=None)
    _EXECC[key] = ent
    return ent


def kernel(**inputs) -> np.ndarray:
    import jax
    src = np.asarray(inputs["src"], np.int32)
    dst = np.asarray(inputs["dst"], np.int32)
    x = np.asarray(inputs["x"])
    base = GATCfg(N=int(x.shape[0]), C=8, IN=int(x.shape[1]))
    cfg, idx, nc = get_built(src, dst, C=8, cfg=base)
    in_maps = host_inputs(inputs, cfg, idx)
    key = (hash(src.tobytes()), hash(dst.tobytes()), cfg.C)
    ent = _get_exec(key, nc, cfg.C)
    C = cfg.C
    concat_in = [np.ascontiguousarray(
        np.concatenate([in_maps[c][nm] for c in range(C)], axis=0))
        for nm in ent["in_names"]]
    hashes = tuple(hash(a.tobytes()) for a in concat_in)
    if ent["argcache"] is None or ent["argcache"][0] != hashes:
        zeros = [np.zeros((C * sh0[0], *sh0[1:]), dt)
                 for sh0, dt in ent["zero_shapes"]]
        args = [jax.device_put(a, ent["sh"]) for a in concat_in + zeros]
        ent["argcache"] = (hashes, args)
    args = ent["argcache"][1]
    outs = ent["f"](*args)
    jax.block_until_ready(outs)
    oi = ent["out_names"].index("out")
    out = np.asarray(outs[oi]).reshape(C, cfg.Nlp, cfg.OUT)
    return out.reshape(-1, cfg.OUT)[:cfg.N].astype(np.float32)
